# revision 13
# baseline (speedup 1.0000x reference)
"""Causal self-attention (B=4, S=2048, D=1024, H=16, hd=64) on 8 TRN2 cores.

Sharding: core c = (batch b = c//2, head-group g = c%2); each core computes
8 heads for one batch. Out-projection partials are summed on host (the only
cross-shard reduction).

Device kernel layout (all matmul contractions have the contracted dim on
SBUF partitions; everything stays transposed so no on-device transposes):
  qT,kT  [64*2heads, S]  = wqkvT-chunk.T @ xT          (stationary weights)
  v_aug  [S-block, 8*65] = xT-chunk.T @ wvT (+ ones col per head for sums)
  sT     [j 128, i 512]  = kT-slice.T @ qT-slice        (2 heads row-packed)
  pT     = exp(sT/8)  bf16 via ACT; diagonal blocks masked by affine_select
  outT   [65, i]        += v_aug.T @ pT   (row 64 accumulates softmax sums)
  attnT  = outT * bcast(1/sums)           (gpsimd partition_broadcast)
  out    [s 128, e]      = attnT-chunk.T @ woutT-chunk  (accum over c-chunks)

QKV work for pair p+1 is emitted between attention pairs so the PE always
has independent matmuls while ACT runs exp (keeps HAM at full clock).
"""
import sys
import os

sys.path.insert(0, "/opt/trn_rl_repo")

import numpy as np
import ml_dtypes
from contextlib import ExitStack

S = 2048
D = 1024
HL = 8          # heads per core
HD = 64
PAIRS = 4       # head pairs per core
NIB = 4         # i-blocks of 512
N_CORES = 8

_CACHE = {}
LAST_EXEC_TIME_NS = None


def _build():
    import concourse.tile as tile
    import concourse.mybir as mybir
    from concourse import bacc

    bf = mybir.dt.bfloat16
    f32 = mybir.dt.float32
    EXP = mybir.ActivationFunctionType.Exp
    GE = mybir.AluOpType.is_ge

    nc = bacc.Bacc("TRN2", target_bir_lowering=False, debug=False,
                   num_devices=N_CORES)
    xT_d = nc.dram_tensor("xT", [D, S], bf, kind="ExternalInput").ap()
    wqkvT_d = nc.dram_tensor("wqkvT", [D, 3 * 512], bf,
                             kind="ExternalInput").ap()
    woutT_d = nc.dram_tensor("woutT", [512, D], bf, kind="ExternalInput").ap()
    out_d = nc.dram_tensor("out", [S, D], f32, kind="ExternalOutput").ap()

    with tile.TileContext(nc) as tc, ExitStack() as ctx:
        sb = ctx.enter_context(tc.tile_pool(name="sb", bufs=1))
        # PSUM: "mm" = 2x [128,1024] (score batches), "ps5" = 4x [128,512]
        # (qkv accum, AV accum, out-proj accum) -> 8 banks total.
        mm = ctx.enter_context(tc.tile_pool(name="mm", bufs=2, space="PSUM"))
        ps5 = ctx.enter_context(tc.tile_pool(name="ps5", bufs=4,
                                             space="PSUM"))
        pp = ctx.enter_context(tc.tile_pool(name="pp", bufs=6))
        rsp = ctx.enter_context(tc.tile_pool(name="rsp", bufs=4))
        bcsp = ctx.enter_context(tc.tile_pool(name="bcsp", bufs=4))
        osbp = ctx.enter_context(tc.tile_pool(name="osbp", bufs=2))

        # ---- persistent SBUF tiles -------------------------------------
        xt = [sb.tile([128, S], bf, tag=f"xt{d}", name=f"xt{d}")
              for d in range(8)]
        wqkv = [sb.tile([128, 1536], bf, tag=f"wqkv{d}", name=f"wqkv{d}")
                for d in range(8)]
        wout = [sb.tile([128, D], bf, tag=f"wout{c}", name=f"wout{c}")
                for c in range(4)]
        qT = [sb.tile([128, S], bf, tag=f"qT{p}", name=f"qT{p}")
              for p in range(PAIRS)]
        kT = [sb.tile([128, S], bf, tag=f"kT{p}", name=f"kT{p}")
              for p in range(PAIRS)]
        vaug = [sb.tile([128, HL, HD + 1], bf, tag=f"vaug{s}",
                        name=f"vaug{s}") for s in range(16)]
        attnT = [sb.tile([128, S], bf, tag=f"attnT{p}", name=f"attnT{p}")
                 for p in range(PAIRS)]

        for d in range(8):
            nc.sync.dma_start(xt[d][:], xT_d[128 * d:128 * (d + 1), :])
            nc.sync.dma_start(wqkv[d][:], wqkvT_d[128 * d:128 * (d + 1), :])
        for c in range(4):
            nc.sync.dma_start(wout[c][:], woutT_d[128 * c:128 * (c + 1), :])
        for s in range(16):
            nc.gpsimd.memset(vaug[s][:], 1.0)

        # ---- emission helpers ------------------------------------------
        def emit_v(sblk):
            ps = ps5.tile([128, 512], f32, tag="ps5", name=f"vps{sblk}")
            for dc in range(8):
                nc.tensor.matmul(ps[:],
                                 lhsT=xt[dc][:, 128 * sblk:128 * (sblk + 1)],
                                 rhs=wqkv[dc][:, 1024:1536],
                                 start=(dc == 0), stop=(dc == 7))
            nc.vector.tensor_copy(
                vaug[sblk][:, :, 0:64],
                ps[:].rearrange("p (h d) -> p h d", h=HL))

        def emit_qk(pair):
            # nb = pair -> q columns, nb = pair + 4 -> k columns
            for nb in (pair, pair + 4):
                dest = qT[pair] if nb < 4 else kT[pair]
                for sc in range(4):
                    ps = ps5.tile([128, 512], f32, tag="ps5",
                                  name=f"qkps{nb}_{sc}")
                    for dc in range(8):
                        nc.tensor.matmul(
                            ps[:],
                            lhsT=wqkv[dc][:, 128 * nb:128 * (nb + 1)],
                            rhs=xt[dc][:, 512 * sc:512 * (sc + 1)],
                            start=(dc == 0), stop=(dc == 7))
                    nc.vector.tensor_copy(dest[:, 512 * sc:512 * (sc + 1)],
                                          ps[:])

        def emit_attn(pair, only_ib=None):
            for ib in range(NIB) if only_ib is None else [only_ib]:
                n_jb = 4 * (ib + 1)
                oA = ps5.tile([65, 512], f32, tag="ps5", name=f"oA{pair}{ib}")
                oB = ps5.tile([65, 512], f32, tag="ps5", name=f"oB{pair}{ib}")
                for jbb in range(0, n_jb, 2):
                    sA = mm.tile([128, 1024], f32, tag="mm",
                                 name=f"sA{pair}{ib}{jbb}")
                    sB = mm.tile([128, 1024], f32, tag="mm",
                                 name=f"sB{pair}{ib}{jbb}")
                    for t in range(2):
                        jb = jbb + t
                        with tc.tile_critical():
                            for h01, sX in ((0, sA), (1, sB)):
                                r0, r1 = 64 * h01, 64 * (h01 + 1)
                                nc.tensor.matmul(
                                    sX[:, 512 * t:512 * (t + 1)],
                                    lhsT=kT[pair][r0:r1,
                                                  128 * jb:128 * (jb + 1)],
                                    rhs=qT[pair][r0:r1,
                                                 512 * ib:512 * (ib + 1)],
                                    start=True, stop=True)
                    pA = pp.tile([128, 1024], bf, tag="pp",
                                 name=f"pA{pair}{ib}{jbb}")
                    pB = pp.tile([128, 1024], bf, tag="pp",
                                 name=f"pB{pair}{ib}{jbb}")
                    nc.scalar.activation(pA[:], sA[:], EXP, scale=0.125)
                    nc.scalar.activation(pB[:], sB[:], EXP, scale=0.125)
                    if jbb >= 4 * ib:     # both jb's diagonal: mask j > i
                        base = 512 * ib - 128 * jbb
                        for pX in (pA, pB):
                            nc.gpsimd.affine_select(
                                out=pX[:].rearrange("p (t i) -> p t i", t=2),
                                in_=pX[:].rearrange("p (t i) -> p t i", t=2),
                                compare_op=GE, fill=0.0, base=base,
                                channel_multiplier=-1,
                                pattern=[[-128, 2], [1, 512]])
                    for t in range(2):
                        jb = jbb + t
                        for h01, (oX, pX) in ((0, (oA, pA)), (1, (oB, pB))):
                            nc.tensor.matmul(
                                oX[:],
                                lhsT=vaug[jb][:, 2 * pair + h01, :],
                                rhs=pX[:, 512 * t:512 * (t + 1)],
                                start=(jb == 0), stop=(jb == n_jb - 1))
                for h01, oX in ((0, oA), (1, oB)):
                    tmp = rsp.tile([1, 512], f32, tag="rtmp",
                                   name=f"rt{pair}{ib}{h01}")
                    nc.vector.tensor_copy(tmp[:], oX[64:65, :])
                    rs = rsp.tile([1, 512], f32, tag="rsp",
                                  name=f"rs{pair}{ib}{h01}")
                    nc.vector.reciprocal_approx_fast(rs[:], tmp[:])
                    bcs = bcsp.tile([64, 512], f32, tag="bcsp",
                                    name=f"bcs{pair}{ib}{h01}")
                    nc.gpsimd.partition_broadcast(bcs[:], rs[:])
                    nc.vector.tensor_mul(
                        attnT[pair][64 * h01:64 * (h01 + 1),
                                    512 * ib:512 * (ib + 1)],
                        oX[0:64, :], bcs[:])

        def emit_outproj(sblk):
            osb = osbp.tile([128, D], f32, tag="osbp", name=f"osb{sblk}")
            for eh in range(2):
                ps = ps5.tile([128, 512], f32, tag="ps5",
                              name=f"ops{sblk}{eh}")
                for cc in range(4):
                    nc.tensor.matmul(
                        ps[:],
                        lhsT=attnT[cc][:, 128 * sblk:128 * (sblk + 1)],
                        rhs=wout[cc][:, 512 * eh:512 * (eh + 1)],
                        start=(cc == 0), stop=(cc == 3))
                nc.vector.tensor_copy(osb[:, 512 * eh:512 * (eh + 1)], ps[:])
            nc.sync.dma_start(out_d[128 * sblk:128 * (sblk + 1), :], osb[:])

        # ---- emission order (== program order for tile deps): vaug[s]
        # must be written before the attention ib that reads it; attnT
        # before the out-proj s-blocks that read it. exp work starts as
        # early as possible; out-proj interleaves with the last pair. -----
        emit_qk(0)
        for ib in range(NIB):
            for sblk in range(4 * ib, 4 * ib + 4):
                emit_v(sblk)
            emit_attn(0, only_ib=ib)
        emit_qk(1)
        emit_attn(1)
        emit_qk(2)
        emit_attn(2)
        emit_qk(3)
        for ib in range(NIB):
            emit_attn(3, only_ib=ib)
            for sblk in range(4 * ib, 4 * ib + 4):
                emit_outproj(sblk)

    nc.compile()
    return nc


def _get_nc():
    if "nc" not in _CACHE:
        _CACHE["nc"] = _build()
    return _CACHE["nc"]


def _shard_inputs(x, w_qkv, w_out):
    bf = ml_dtypes.bfloat16
    in_maps = []
    for c in range(N_CORES):
        b, g = divmod(c, 2)
        xT = np.ascontiguousarray(x[b].T).astype(bf)
        wq = w_qkv[512 * g:512 * (g + 1)]
        wk = w_qkv[1024 + 512 * g:1024 + 512 * (g + 1)]
        wv = w_qkv[2048 + 512 * g:2048 + 512 * (g + 1)]
        wqkvT = np.ascontiguousarray(
            np.concatenate([wq, wk, wv], axis=0).T).astype(bf)
        woutT = np.ascontiguousarray(w_out[:, 512 * g:512 * (g + 1)].T
                                     ).astype(bf)
        in_maps.append({"xT": xT, "wqkvT": wqkvT, "woutT": woutT})
    return in_maps


def kernel(x, w_qkv, w_out):
    global LAST_EXEC_TIME_NS
    from concourse.bass_utils import run_bass_kernel_spmd

    nc = _get_nc()
    in_maps = _shard_inputs(np.asarray(x, dtype=np.float32),
                            np.asarray(w_qkv, dtype=np.float32),
                            np.asarray(w_out, dtype=np.float32))
    trace = bool(int(os.environ.get("KBENCH_TRACE", "0")))
    res = run_bass_kernel_spmd(nc, in_maps, list(range(N_CORES)), trace=trace)
    LAST_EXEC_TIME_NS = res.exec_time_ns
    out = np.empty((4, S, D), dtype=np.float32)
    for b in range(4):
        out[b] = res.results[2 * b]["out"] + res.results[2 * b + 1]["out"]
    return out


# revision 14
# speedup vs baseline: 1.7619x; 1.7619x over previous
"""Causal self-attention (B=4, S=2048, D=1024, H=16, hd=64) on 8 TRN2 cores.

Sharding: core c = (batch b = c//2, head-group g = c%2); each core computes
8 heads for one batch. Out-projection partials are summed on host (the only
cross-shard reduction).

Device kernel layout (all matmul contractions have the contracted dim on
SBUF partitions; everything stays transposed so no on-device transposes):
  qT,kT  [64*2heads, S]  = wqkvT-chunk.T @ xT          (stationary weights)
  v_aug  [S-block, 8*65] = xT-chunk.T @ wvT (+ ones col per head for sums)
  sT     [j 128, i 512]  = kT-slice.T @ qT-slice        (2 heads row-packed)
  pT     = exp(sT/8)  bf16 via ACT; diagonal blocks masked by affine_select
  outT   [65, i]        += v_aug.T @ pT   (row 64 accumulates softmax sums)
  attnT  = outT * bcast(1/sums)           (gpsimd partition_broadcast)
  out    [s 128, e]      = attnT-chunk.T @ woutT-chunk  (accum over c-chunks)

QKV work for pair p+1 is emitted between attention pairs so the PE always
has independent matmuls while ACT runs exp (keeps HAM at full clock).
"""
import sys
import os

sys.path.insert(0, "/opt/trn_rl_repo")

import numpy as np
import ml_dtypes
from contextlib import ExitStack

S = 2048
D = 1024
HL = 8          # heads per core
HD = 64
PAIRS = 4       # head pairs per core
NIB = 4         # i-blocks of 512
N_CORES = 8

_CACHE = {}
LAST_EXEC_TIME_NS = None


def _build():
    import concourse.tile as tile
    import concourse.mybir as mybir
    from concourse import bacc

    bf = mybir.dt.bfloat16
    f32 = mybir.dt.float32
    EXP = mybir.ActivationFunctionType.Exp
    GE = mybir.AluOpType.is_ge

    nc = bacc.Bacc("TRN2", target_bir_lowering=False, debug=False,
                   num_devices=N_CORES)
    xT_d = nc.dram_tensor("xT", [D, S], bf, kind="ExternalInput").ap()
    wqkvT_d = nc.dram_tensor("wqkvT", [D, 3 * 512], bf,
                             kind="ExternalInput").ap()
    woutT_d = nc.dram_tensor("woutT", [512, D], bf, kind="ExternalInput").ap()
    out_d = nc.dram_tensor("out", [S, D], f32, kind="ExternalOutput").ap()

    with tile.TileContext(nc) as tc, ExitStack() as ctx:
        sb = ctx.enter_context(tc.tile_pool(name="sb", bufs=1))
        # PSUM: "mm" = 2x [128,1024] (score batches), "ps5" = 4x [128,512]
        # (qkv accum, AV accum, out-proj accum) -> 8 banks total.
        mm = ctx.enter_context(tc.tile_pool(name="mm", bufs=2, space="PSUM"))
        ps5 = ctx.enter_context(tc.tile_pool(name="ps5", bufs=4,
                                             space="PSUM"))
        pp = ctx.enter_context(tc.tile_pool(name="pp", bufs=6))
        rsp = ctx.enter_context(tc.tile_pool(name="rsp", bufs=4))
        bcsp = ctx.enter_context(tc.tile_pool(name="bcsp", bufs=4))
        osbp = ctx.enter_context(tc.tile_pool(name="osbp", bufs=2))

        # ---- persistent SBUF tiles -------------------------------------
        xt = [sb.tile([128, S], bf, tag=f"xt{d}", name=f"xt{d}")
              for d in range(8)]
        wqkv = [sb.tile([128, 1536], bf, tag=f"wqkv{d}", name=f"wqkv{d}")
                for d in range(8)]
        wout = [sb.tile([128, D], bf, tag=f"wout{c}", name=f"wout{c}")
                for c in range(4)]
        qT = [sb.tile([128, S], bf, tag=f"qT{p}", name=f"qT{p}")
              for p in range(PAIRS)]
        kT = [sb.tile([128, S], bf, tag=f"kT{p}", name=f"kT{p}")
              for p in range(PAIRS)]
        vaug = [sb.tile([128, HL, HD + 1], bf, tag=f"vaug{s}",
                        name=f"vaug{s}") for s in range(16)]
        attnT = [sb.tile([128, S], bf, tag=f"attnT{p}", name=f"attnT{p}")
                 for p in range(PAIRS)]

        for d in range(8):
            nc.sync.dma_start(xt[d][:], xT_d[128 * d:128 * (d + 1), :])
            nc.sync.dma_start(wqkv[d][:], wqkvT_d[128 * d:128 * (d + 1), :])
        for c in range(4):
            nc.sync.dma_start(wout[c][:], woutT_d[128 * c:128 * (c + 1), :])
        for s in range(16):
            nc.gpsimd.memset(vaug[s][:], 1.0)

        # ---- emission helpers ------------------------------------------
        def emit_v(sblk):
            ps = ps5.tile([128, 512], f32, tag="ps5", name=f"vps{sblk}")
            for dc in range(8):
                nc.tensor.matmul(ps[:],
                                 lhsT=xt[dc][:, 128 * sblk:128 * (sblk + 1)],
                                 rhs=wqkv[dc][:, 1024:1536],
                                 start=(dc == 0), stop=(dc == 7))
            nc.vector.tensor_copy(
                vaug[sblk][:, :, 0:64],
                ps[:].rearrange("p (h d) -> p h d", h=HL))

        def emit_qk(pair):
            # nb = pair -> q columns, nb = pair + 4 -> k columns
            for nb in (pair, pair + 4):
                dest = qT[pair] if nb < 4 else kT[pair]
                for sc in range(4):
                    ps = ps5.tile([128, 512], f32, tag="ps5",
                                  name=f"qkps{nb}_{sc}")
                    for dc in range(8):
                        nc.tensor.matmul(
                            ps[:],
                            lhsT=wqkv[dc][:, 128 * nb:128 * (nb + 1)],
                            rhs=xt[dc][:, 512 * sc:512 * (sc + 1)],
                            start=(dc == 0), stop=(dc == 7))
                    nc.vector.tensor_copy(dest[:, 512 * sc:512 * (sc + 1)],
                                          ps[:])

        def emit_attn(pair, only_ib=None):
            for ib in range(NIB) if only_ib is None else [only_ib]:
                n_jb = 4 * (ib + 1)
                oA = ps5.tile([65, 512], f32, tag="ps5", name=f"oA{pair}{ib}")
                oB = ps5.tile([65, 512], f32, tag="ps5", name=f"oB{pair}{ib}")
                for jbb in range(0, n_jb, 2):
                    sA = mm.tile([128, 1024], f32, tag="mm",
                                 name=f"sA{pair}{ib}{jbb}")
                    sB = mm.tile([128, 1024], f32, tag="mm",
                                 name=f"sB{pair}{ib}{jbb}")
                    for t in range(2):
                        jb = jbb + t
                        for h01, sX in ((0, sA), (1, sB)):
                            r0, r1 = 64 * h01, 64 * (h01 + 1)
                            nc.tensor.matmul(
                                sX[:, 512 * t:512 * (t + 1)],
                                lhsT=kT[pair][r0:r1,
                                              128 * jb:128 * (jb + 1)],
                                rhs=qT[pair][r0:r1,
                                             512 * ib:512 * (ib + 1)],
                                start=True, stop=True)
                    pA = pp.tile([128, 1024], bf, tag="pp",
                                 name=f"pA{pair}{ib}{jbb}")
                    pB = pp.tile([128, 1024], bf, tag="pp",
                                 name=f"pB{pair}{ib}{jbb}")
                    nc.scalar.activation(pA[:], sA[:], EXP, scale=0.125)
                    nc.scalar.activation(pB[:], sB[:], EXP, scale=0.125)
                    if jbb >= 4 * ib:     # both jb's diagonal: mask j > i
                        base = 512 * ib - 128 * jbb
                        for pX in (pA, pB):
                            nc.gpsimd.affine_select(
                                out=pX[:].rearrange("p (t i) -> p t i", t=2),
                                in_=pX[:].rearrange("p (t i) -> p t i", t=2),
                                compare_op=GE, fill=0.0, base=base,
                                channel_multiplier=-1,
                                pattern=[[-128, 2], [1, 512]])
                    for t in range(2):
                        jb = jbb + t
                        for h01, (oX, pX) in ((0, (oA, pA)), (1, (oB, pB))):
                            nc.tensor.matmul(
                                oX[:],
                                lhsT=vaug[jb][:, 2 * pair + h01, :],
                                rhs=pX[:, 512 * t:512 * (t + 1)],
                                start=(jb == 0), stop=(jb == n_jb - 1))
                for h01, oX in ((0, oA), (1, oB)):
                    tmp = rsp.tile([1, 512], f32, tag="rtmp",
                                   name=f"rt{pair}{ib}{h01}")
                    nc.vector.tensor_copy(tmp[:], oX[64:65, :])
                    rs = rsp.tile([1, 512], f32, tag="rsp",
                                  name=f"rs{pair}{ib}{h01}")
                    nc.vector.reciprocal_approx_fast(rs[:], tmp[:])
                    bcs = bcsp.tile([64, 512], f32, tag="bcsp",
                                    name=f"bcs{pair}{ib}{h01}")
                    nc.gpsimd.partition_broadcast(bcs[:], rs[:])
                    nc.vector.tensor_mul(
                        attnT[pair][64 * h01:64 * (h01 + 1),
                                    512 * ib:512 * (ib + 1)],
                        oX[0:64, :], bcs[:])

        def emit_outproj(sblk):
            osb = osbp.tile([128, D], f32, tag="osbp", name=f"osb{sblk}")
            for eh in range(2):
                ps = ps5.tile([128, 512], f32, tag="ps5",
                              name=f"ops{sblk}{eh}")
                for cc in range(4):
                    nc.tensor.matmul(
                        ps[:],
                        lhsT=attnT[cc][:, 128 * sblk:128 * (sblk + 1)],
                        rhs=wout[cc][:, 512 * eh:512 * (eh + 1)],
                        start=(cc == 0), stop=(cc == 3))
                nc.vector.tensor_copy(osb[:, 512 * eh:512 * (eh + 1)], ps[:])
            nc.sync.dma_start(out_d[128 * sblk:128 * (sblk + 1), :], osb[:])

        # ---- emission order (== program order for tile deps): vaug[s]
        # must be written before the attention ib that reads it; attnT
        # before the out-proj s-blocks that read it. exp work starts as
        # early as possible; out-proj interleaves with the last pair. -----
        emit_qk(0)
        for ib in range(NIB):
            for sblk in range(4 * ib, 4 * ib + 4):
                emit_v(sblk)
            emit_attn(0, only_ib=ib)
        emit_qk(1)
        emit_attn(1)
        emit_qk(2)
        emit_attn(2)
        emit_qk(3)
        for ib in range(NIB):
            emit_attn(3, only_ib=ib)
            for sblk in range(4 * ib, 4 * ib + 4):
                emit_outproj(sblk)

    nc.compile()
    return nc


def _get_nc():
    if "nc" not in _CACHE:
        _CACHE["nc"] = _build()
    return _CACHE["nc"]


def _shard_inputs(x, w_qkv, w_out):
    bf = ml_dtypes.bfloat16
    in_maps = []
    for c in range(N_CORES):
        b, g = divmod(c, 2)
        xT = np.ascontiguousarray(x[b].T).astype(bf)
        wq = w_qkv[512 * g:512 * (g + 1)]
        wk = w_qkv[1024 + 512 * g:1024 + 512 * (g + 1)]
        wv = w_qkv[2048 + 512 * g:2048 + 512 * (g + 1)]
        wqkvT = np.ascontiguousarray(
            np.concatenate([wq, wk, wv], axis=0).T).astype(bf)
        woutT = np.ascontiguousarray(w_out[:, 512 * g:512 * (g + 1)].T
                                     ).astype(bf)
        in_maps.append({"xT": xT, "wqkvT": wqkvT, "woutT": woutT})
    return in_maps


def kernel(x, w_qkv, w_out):
    global LAST_EXEC_TIME_NS
    from concourse.bass_utils import run_bass_kernel_spmd

    nc = _get_nc()
    in_maps = _shard_inputs(np.asarray(x, dtype=np.float32),
                            np.asarray(w_qkv, dtype=np.float32),
                            np.asarray(w_out, dtype=np.float32))
    trace = bool(int(os.environ.get("KBENCH_TRACE", "0")))
    res = run_bass_kernel_spmd(nc, in_maps, list(range(N_CORES)), trace=trace)
    LAST_EXEC_TIME_NS = res.exec_time_ns
    out = np.empty((4, S, D), dtype=np.float32)
    for b in range(4):
        out[b] = res.results[2 * b]["out"] + res.results[2 * b + 1]["out"]
    return out


# revision 16
# speedup vs baseline: 1.9144x; 1.0865x over previous
"""Causal self-attention (B=4, S=2048, D=1024, H=16, hd=64) on 8 TRN2 cores.

Sharding: core c = (batch b = c//2, head-group g = c%2); each core computes
8 heads for one batch. Out-projection partials are summed on host (the only
cross-shard reduction).

Device kernel layout (all matmul contractions have the contracted dim on
SBUF partitions; everything stays transposed so no on-device transposes):
  qT,kT  [64*2heads, S]  = wqkvT-chunk.T @ xT          (stationary weights)
  v_aug  [S-block, 8*65] = xT-chunk.T @ wvT (+ ones col per head for sums)
  sT     [j 128, i 512]  = kT-slice.T @ qT-slice        (2 heads row-packed)
  pT     = exp(sT/8)  bf16 via ACT; diagonal blocks masked by affine_select
  outT   [65, i]        += v_aug.T @ pT   (row 64 accumulates softmax sums)
  attnT  = outT * bcast(1/sums)           (gpsimd partition_broadcast)
  out    [s 128, e]      = attnT-chunk.T @ woutT-chunk  (accum over c-chunks)

QKV work for pair p+1 is emitted between attention pairs so the PE always
has independent matmuls while ACT runs exp (keeps HAM at full clock).
"""
import sys
import os

sys.path.insert(0, "/opt/trn_rl_repo")

import numpy as np
import ml_dtypes
from contextlib import ExitStack

S = 2048
D = 1024
HL = 8          # heads per core
HD = 64
PAIRS = 4       # head pairs per core
NIB = 4         # i-blocks of 512
N_CORES = 8

_CACHE = {}
LAST_EXEC_TIME_NS = None


def _build():
    import concourse.tile as tile
    import concourse.mybir as mybir
    from concourse import bacc

    bf = mybir.dt.bfloat16
    f32 = mybir.dt.float32
    EXP = mybir.ActivationFunctionType.Exp
    GE = mybir.AluOpType.is_ge

    nc = bacc.Bacc("TRN2", target_bir_lowering=False, debug=False,
                   num_devices=N_CORES)
    xT_d = nc.dram_tensor("xT", [D, S], bf, kind="ExternalInput").ap()
    wqkvT_d = nc.dram_tensor("wqkvT", [D, 3 * 512], bf,
                             kind="ExternalInput").ap()
    woutT_d = nc.dram_tensor("woutT", [512, D], bf, kind="ExternalInput").ap()
    out_d = nc.dram_tensor("out", [S, D], f32, kind="ExternalOutput").ap()

    with tile.TileContext(nc) as tc, ExitStack() as ctx:
        sb = ctx.enter_context(tc.tile_pool(name="sb", bufs=1))
        # PSUM: "mm" = 2x [128,1024] (score batches), "ps5" = 4x [128,512]
        # (qkv accum, AV accum, out-proj accum) -> 8 banks total.
        mm = ctx.enter_context(tc.tile_pool(name="mm", bufs=2, space="PSUM"))
        ps5 = ctx.enter_context(tc.tile_pool(name="ps5", bufs=4,
                                             space="PSUM"))
        pp = ctx.enter_context(tc.tile_pool(name="pp", bufs=6))
        rsp = ctx.enter_context(tc.tile_pool(name="rsp", bufs=4))
        bcsp = ctx.enter_context(tc.tile_pool(name="bcsp", bufs=4))
        osbp = ctx.enter_context(tc.tile_pool(name="osbp", bufs=2))

        # ---- persistent SBUF tiles -------------------------------------
        xt = [sb.tile([128, S], bf, tag=f"xt{d}", name=f"xt{d}")
              for d in range(8)]
        wqkv = [sb.tile([128, 1536], bf, tag=f"wqkv{d}", name=f"wqkv{d}")
                for d in range(8)]
        wout = [sb.tile([128, D], bf, tag=f"wout{c}", name=f"wout{c}")
                for c in range(4)]
        qT = [sb.tile([128, S], bf, tag=f"qT{p}", name=f"qT{p}")
              for p in range(PAIRS)]
        kT = [sb.tile([128, S], bf, tag=f"kT{p}", name=f"kT{p}")
              for p in range(PAIRS)]
        vaug = [sb.tile([128, HL, HD + 1], bf, tag=f"vaug{s}",
                        name=f"vaug{s}") for s in range(16)]
        attnT = [sb.tile([128, S], bf, tag=f"attnT{p}", name=f"attnT{p}")
                 for p in range(PAIRS)]

        for d in range(8):
            nc.sync.dma_start(xt[d][:], xT_d[128 * d:128 * (d + 1), :])
            nc.sync.dma_start(wqkv[d][:], wqkvT_d[128 * d:128 * (d + 1), :])
        for c in range(4):
            nc.sync.dma_start(wout[c][:], woutT_d[128 * c:128 * (c + 1), :])
        for s in range(16):
            nc.gpsimd.memset(vaug[s][:], 1.0)
        # causal masks for the 4 diagonal offsets: keep where i >= 128*m + j
        masks = [sb.tile([128, 512], bf, tag=f"mask{m}", name=f"mask{m}")
                 for m in range(4)]
        for m in range(4):
            nc.gpsimd.memset(masks[m][:], 1.0)
            nc.gpsimd.affine_select(
                out=masks[m][:], in_=masks[m][:], compare_op=GE, fill=0.0,
                base=-128 * m, channel_multiplier=-1, pattern=[[1, 512]])

        # ---- emission helpers ------------------------------------------
        def emit_v(sblk):
            ps = ps5.tile([128, 512], f32, tag="ps5", name=f"vps{sblk}")
            for dc in range(8):
                nc.tensor.matmul(ps[:],
                                 lhsT=xt[dc][:, 128 * sblk:128 * (sblk + 1)],
                                 rhs=wqkv[dc][:, 1024:1536],
                                 start=(dc == 0), stop=(dc == 7))
            nc.vector.tensor_copy(
                vaug[sblk][:, :, 0:64],
                ps[:].rearrange("p (h d) -> p h d", h=HL))

        def emit_qk(pair):
            # nb = pair -> q columns, nb = pair + 4 -> k columns
            for nb in (pair, pair + 4):
                dest = qT[pair] if nb < 4 else kT[pair]
                for sc in range(4):
                    ps = ps5.tile([128, 512], f32, tag="ps5",
                                  name=f"qkps{nb}_{sc}")
                    for dc in range(8):
                        nc.tensor.matmul(
                            ps[:],
                            lhsT=wqkv[dc][:, 128 * nb:128 * (nb + 1)],
                            rhs=xt[dc][:, 512 * sc:512 * (sc + 1)],
                            start=(dc == 0), stop=(dc == 7))
                    nc.vector.tensor_copy(dest[:, 512 * sc:512 * (sc + 1)],
                                          ps[:])

        def emit_attn(pair, only_ib=None):
            for ib in range(NIB) if only_ib is None else [only_ib]:
                n_jb = 4 * (ib + 1)
                oA = ps5.tile([65, 512], f32, tag="ps5", name=f"oA{pair}{ib}")
                oB = ps5.tile([65, 512], f32, tag="ps5", name=f"oB{pair}{ib}")
                for jb in range(n_jb):
                    # both heads' scores in ONE psum tile so the scheduler
                    # keeps the two row-group matmuls adjacent (they then
                    # run concurrently on disjoint array row halves)
                    s2 = mm.tile([128, 1024], f32, tag="mm",
                                 name=f"s2_{pair}{ib}{jb}")
                    for h01 in range(2):
                        r0, r1 = 64 * h01, 64 * (h01 + 1)
                        nc.tensor.matmul(
                            s2[:, 512 * h01:512 * (h01 + 1)],
                            lhsT=kT[pair][r0:r1, 128 * jb:128 * (jb + 1)],
                            rhs=qT[pair][r0:r1, 512 * ib:512 * (ib + 1)],
                            start=True, stop=True)
                    pX = pp.tile([128, 1024], bf, tag="pp",
                                 name=f"pX{pair}{ib}{jb}")
                    nc.scalar.activation(pX[:], s2[:], EXP, scale=0.125)
                    if jb >= 4 * ib:      # diagonal block: zero j > i
                        m = jb - 4 * ib
                        v3 = pX[:].rearrange("p (h i) -> p h i", h=2)
                        nc.vector.tensor_mul(
                            v3, v3,
                            masks[m][:].unsqueeze(1).broadcast_to(
                                [128, 2, 512]))
                    for h01, oX in ((0, oA), (1, oB)):
                        nc.tensor.matmul(
                            oX[:],
                            lhsT=vaug[jb][:, 2 * pair + h01, :],
                            rhs=pX[:, 512 * h01:512 * (h01 + 1)],
                            start=(jb == 0), stop=(jb == n_jb - 1))
                for h01, oX in ((0, oA), (1, oB)):
                    tmp = rsp.tile([1, 512], f32, tag="rtmp",
                                   name=f"rt{pair}{ib}{h01}")
                    nc.vector.tensor_copy(tmp[:], oX[64:65, :])
                    rs = rsp.tile([1, 512], f32, tag="rsp",
                                  name=f"rs{pair}{ib}{h01}")
                    nc.vector.reciprocal_approx_fast(rs[:], tmp[:])
                    bcs = bcsp.tile([64, 512], f32, tag="bcsp",
                                    name=f"bcs{pair}{ib}{h01}")
                    nc.gpsimd.partition_broadcast(bcs[:], rs[:])
                    nc.vector.tensor_mul(
                        attnT[pair][64 * h01:64 * (h01 + 1),
                                    512 * ib:512 * (ib + 1)],
                        oX[0:64, :], bcs[:])

        def emit_outproj(sblk):
            osb = osbp.tile([128, D], f32, tag="osbp", name=f"osb{sblk}")
            for eh in range(2):
                ps = ps5.tile([128, 512], f32, tag="ps5",
                              name=f"ops{sblk}{eh}")
                for cc in range(4):
                    nc.tensor.matmul(
                        ps[:],
                        lhsT=attnT[cc][:, 128 * sblk:128 * (sblk + 1)],
                        rhs=wout[cc][:, 512 * eh:512 * (eh + 1)],
                        start=(cc == 0), stop=(cc == 3))
                nc.vector.tensor_copy(osb[:, 512 * eh:512 * (eh + 1)], ps[:])
            nc.sync.dma_start(out_d[128 * sblk:128 * (sblk + 1), :], osb[:])

        # ---- emission order (== program order for tile deps): vaug[s]
        # must be written before the attention ib that reads it; attnT
        # before the out-proj s-blocks that read it. exp work starts as
        # early as possible; out-proj interleaves with the last pair. -----
        emit_qk(0)
        for ib in range(NIB):
            for sblk in range(4 * ib, 4 * ib + 4):
                emit_v(sblk)
            emit_attn(0, only_ib=ib)
        emit_qk(1)
        emit_attn(1)
        emit_qk(2)
        emit_attn(2)
        emit_qk(3)
        for ib in range(NIB):
            emit_attn(3, only_ib=ib)
            for sblk in range(4 * ib, 4 * ib + 4):
                emit_outproj(sblk)

    nc.compile()
    return nc


def _get_nc():
    if "nc" not in _CACHE:
        _CACHE["nc"] = _build()
    return _CACHE["nc"]


def _shard_inputs(x, w_qkv, w_out):
    bf = ml_dtypes.bfloat16
    in_maps = []
    for c in range(N_CORES):
        b, g = divmod(c, 2)
        xT = np.ascontiguousarray(x[b].T).astype(bf)
        wq = w_qkv[512 * g:512 * (g + 1)]
        wk = w_qkv[1024 + 512 * g:1024 + 512 * (g + 1)]
        wv = w_qkv[2048 + 512 * g:2048 + 512 * (g + 1)]
        wqkvT = np.ascontiguousarray(
            np.concatenate([wq, wk, wv], axis=0).T).astype(bf)
        woutT = np.ascontiguousarray(w_out[:, 512 * g:512 * (g + 1)].T
                                     ).astype(bf)
        in_maps.append({"xT": xT, "wqkvT": wqkvT, "woutT": woutT})
    return in_maps


def kernel(x, w_qkv, w_out):
    global LAST_EXEC_TIME_NS
    from concourse.bass_utils import run_bass_kernel_spmd

    nc = _get_nc()
    in_maps = _shard_inputs(np.asarray(x, dtype=np.float32),
                            np.asarray(w_qkv, dtype=np.float32),
                            np.asarray(w_out, dtype=np.float32))
    trace = bool(int(os.environ.get("KBENCH_TRACE", "0")))
    res = run_bass_kernel_spmd(nc, in_maps, list(range(N_CORES)), trace=trace)
    LAST_EXEC_TIME_NS = res.exec_time_ns
    out = np.empty((4, S, D), dtype=np.float32)
    for b in range(4):
        out[b] = res.results[2 * b]["out"] + res.results[2 * b + 1]["out"]
    return out


# revision 17
# speedup vs baseline: 1.9544x; 1.0209x over previous
"""Causal self-attention (B=4, S=2048, D=1024, H=16, hd=64) on 8 TRN2 cores.

Sharding: core c = (batch b = c//2, head-group g = c%2); each core computes
8 heads for one batch. Out-projection partials are summed on host (the only
cross-shard reduction).

Device kernel layout (all matmul contractions have the contracted dim on
SBUF partitions; everything stays transposed so no on-device transposes):
  qT,kT  [64*2heads, S]  = wqkvT-chunk.T @ xT          (stationary weights)
  v_aug  [S-block, 8*65] = xT-chunk.T @ wvT (+ ones col per head for sums)
  sT     [j 128, i 512]  = kT-slice.T @ qT-slice        (2 heads row-packed)
  pT     = exp(sT/8)  bf16 via ACT; diagonal blocks masked by affine_select
  outT   [65, i]        += v_aug.T @ pT   (row 64 accumulates softmax sums)
  attnT  = outT * bcast(1/sums)           (gpsimd partition_broadcast)
  out    [s 128, e]      = attnT-chunk.T @ woutT-chunk  (accum over c-chunks)

QKV work for pair p+1 is emitted between attention pairs so the PE always
has independent matmuls while ACT runs exp (keeps HAM at full clock).
"""
import sys
import os

sys.path.insert(0, "/opt/trn_rl_repo")

import numpy as np
import ml_dtypes
from contextlib import ExitStack

S = 2048
D = 1024
HL = 8          # heads per core
HD = 64
PAIRS = 4       # head pairs per core
NIB = 4         # i-blocks of 512
N_CORES = 8

_CACHE = {}
LAST_EXEC_TIME_NS = None


def _build():
    import concourse.tile as tile
    import concourse.mybir as mybir
    from concourse import bacc

    bf = mybir.dt.bfloat16
    f32 = mybir.dt.float32
    EXP = mybir.ActivationFunctionType.Exp
    GE = mybir.AluOpType.is_ge

    nc = bacc.Bacc("TRN2", target_bir_lowering=False, debug=False,
                   num_devices=N_CORES)
    xT_d = nc.dram_tensor("xT", [D, S], bf, kind="ExternalInput").ap()
    wqkvT_d = nc.dram_tensor("wqkvT", [D, 3 * 512], bf,
                             kind="ExternalInput").ap()
    woutT_d = nc.dram_tensor("woutT", [512, D], bf, kind="ExternalInput").ap()
    out_d = nc.dram_tensor("out", [S, D], f32, kind="ExternalOutput").ap()

    with tile.TileContext(nc) as tc, ExitStack() as ctx:
        sb = ctx.enter_context(tc.tile_pool(name="sb", bufs=1))
        # PSUM: "mm" = 2x [128,1024] (score batches), "ps5" = 4x [128,512]
        # (qkv accum, AV accum, out-proj accum) -> 8 banks total.
        mm = ctx.enter_context(tc.tile_pool(name="mm", bufs=2, space="PSUM"))
        ps5 = ctx.enter_context(tc.tile_pool(name="ps5", bufs=4,
                                             space="PSUM"))
        pp = ctx.enter_context(tc.tile_pool(name="pp", bufs=6))
        rsp = ctx.enter_context(tc.tile_pool(name="rsp", bufs=4))
        bcsp = ctx.enter_context(tc.tile_pool(name="bcsp", bufs=4))
        osbp = ctx.enter_context(tc.tile_pool(name="osbp", bufs=2))

        # ---- persistent SBUF tiles -------------------------------------
        xt = [sb.tile([128, S], bf, tag=f"xt{d}", name=f"xt{d}")
              for d in range(8)]
        wqkv = [sb.tile([128, 1536], bf, tag=f"wqkv{d}", name=f"wqkv{d}")
                for d in range(8)]
        wout = [sb.tile([128, D], bf, tag=f"wout{c}", name=f"wout{c}")
                for c in range(4)]
        qT = [sb.tile([128, S], bf, tag=f"qT{p}", name=f"qT{p}")
              for p in range(PAIRS)]
        kT = [sb.tile([128, S], bf, tag=f"kT{p}", name=f"kT{p}")
              for p in range(PAIRS)]
        vaug = [sb.tile([128, HL, HD + 1], bf, tag=f"vaug{s}",
                        name=f"vaug{s}") for s in range(16)]
        attnT = [sb.tile([128, S], bf, tag=f"attnT{p}", name=f"attnT{p}")
                 for p in range(PAIRS)]

        for d in range(8):
            nc.sync.dma_start(xt[d][:], xT_d[128 * d:128 * (d + 1), :])
            nc.sync.dma_start(wqkv[d][:], wqkvT_d[128 * d:128 * (d + 1), :])
        for c in range(4):
            nc.sync.dma_start(wout[c][:], woutT_d[128 * c:128 * (c + 1), :])
        for s in range(16):
            nc.gpsimd.memset(vaug[s][:], 1.0)
        # causal masks for the 4 diagonal offsets: keep where i >= 128*m + j
        masks = [sb.tile([128, 512], bf, tag=f"mask{m}", name=f"mask{m}")
                 for m in range(4)]
        for m in range(4):
            nc.gpsimd.memset(masks[m][:], 1.0)
            nc.gpsimd.affine_select(
                out=masks[m][:], in_=masks[m][:], compare_op=GE, fill=0.0,
                base=-128 * m, channel_multiplier=-1, pattern=[[1, 512]])

        # ---- emission helpers ------------------------------------------
        def emit_v(sblk):
            ps = ps5.tile([128, 512], f32, tag="ps5", name=f"vps{sblk}")
            for dc in range(8):
                nc.tensor.matmul(ps[:],
                                 lhsT=xt[dc][:, 128 * sblk:128 * (sblk + 1)],
                                 rhs=wqkv[dc][:, 1024:1536],
                                 start=(dc == 0), stop=(dc == 7))
            nc.vector.tensor_copy(
                vaug[sblk][:, :, 0:64],
                ps[:].rearrange("p (h d) -> p h d", h=HL))

        def emit_qk(pair):
            # nb = pair -> q columns, nb = pair + 4 -> k columns
            for nb in (pair, pair + 4):
                dest = qT[pair] if nb < 4 else kT[pair]
                for sc in range(4):
                    ps = ps5.tile([128, 512], f32, tag="ps5",
                                  name=f"qkps{nb}_{sc}")
                    for dc in range(8):
                        nc.tensor.matmul(
                            ps[:],
                            lhsT=wqkv[dc][:, 128 * nb:128 * (nb + 1)],
                            rhs=xt[dc][:, 512 * sc:512 * (sc + 1)],
                            start=(dc == 0), stop=(dc == 7))
                    nc.vector.tensor_copy(dest[:, 512 * sc:512 * (sc + 1)],
                                          ps[:])

        def emit_attn(pair, only_ib=None):
            for ib in range(NIB) if only_ib is None else [only_ib]:
                n_jb = 4 * (ib + 1)
                oA = ps5.tile([65, 512], f32, tag="ps5", name=f"oA{pair}{ib}")
                oB = ps5.tile([65, 512], f32, tag="ps5", name=f"oB{pair}{ib}")
                for jb in range(n_jb):
                    # i-offset into the 512-block below which this j-block
                    # is fully masked (diagonal blocks only)
                    off = max(0, 128 * (jb - 4 * ib))
                    w = 512 - off
                    # both heads' scores in ONE psum tile so the scheduler
                    # keeps the two row-group matmuls adjacent (they then
                    # run concurrently on disjoint array row halves)
                    s2 = mm.tile([128, 1024], f32, tag="mm",
                                 name=f"s2_{pair}{ib}{jb}")
                    for h01 in range(2):
                        r0, r1 = 64 * h01, 64 * (h01 + 1)
                        nc.tensor.matmul(
                            s2[:, 512 * h01 + off:512 * (h01 + 1)],
                            lhsT=kT[pair][r0:r1, 128 * jb:128 * (jb + 1)],
                            rhs=qT[pair][r0:r1,
                                         512 * ib + off:512 * (ib + 1)],
                            start=True, stop=True)
                    pX = pp.tile([128, 1024], bf, tag="pp",
                                 name=f"pX{pair}{ib}{jb}")
                    s3 = s2[:].rearrange("p (h i) -> p h i", h=2)
                    p3 = pX[:].rearrange("p (h i) -> p h i", h=2)
                    nc.scalar.activation(p3[:, :, off:512], s3[:, :, off:512],
                                         EXP, scale=0.125)
                    if jb >= 4 * ib:      # diagonal block: zero j > i
                        m = jb - 4 * ib
                        nc.vector.tensor_mul(
                            p3[:, :, off:512], p3[:, :, off:512],
                            masks[m][:, off:512].unsqueeze(1).broadcast_to(
                                [128, 2, w]))
                    for h01, oX in ((0, oA), (1, oB)):
                        nc.tensor.matmul(
                            oX[:, off:512],
                            lhsT=vaug[jb][:, 2 * pair + h01, :],
                            rhs=pX[:, 512 * h01 + off:512 * (h01 + 1)],
                            start=(jb == 0), stop=(jb == n_jb - 1))
                for h01, oX in ((0, oA), (1, oB)):
                    tmp = rsp.tile([1, 512], f32, tag="rtmp",
                                   name=f"rt{pair}{ib}{h01}")
                    nc.vector.tensor_copy(tmp[:], oX[64:65, :])
                    rs = rsp.tile([1, 512], f32, tag="rsp",
                                  name=f"rs{pair}{ib}{h01}")
                    nc.vector.reciprocal_approx_fast(rs[:], tmp[:])
                    bcs = bcsp.tile([64, 512], f32, tag="bcsp",
                                    name=f"bcs{pair}{ib}{h01}")
                    nc.gpsimd.partition_broadcast(bcs[:], rs[:])
                    nc.vector.tensor_mul(
                        attnT[pair][64 * h01:64 * (h01 + 1),
                                    512 * ib:512 * (ib + 1)],
                        oX[0:64, :], bcs[:])

        def emit_outproj(sblk):
            osb = osbp.tile([128, D], f32, tag="osbp", name=f"osb{sblk}")
            for eh in range(2):
                ps = ps5.tile([128, 512], f32, tag="ps5",
                              name=f"ops{sblk}{eh}")
                for cc in range(4):
                    nc.tensor.matmul(
                        ps[:],
                        lhsT=attnT[cc][:, 128 * sblk:128 * (sblk + 1)],
                        rhs=wout[cc][:, 512 * eh:512 * (eh + 1)],
                        start=(cc == 0), stop=(cc == 3))
                nc.vector.tensor_copy(osb[:, 512 * eh:512 * (eh + 1)], ps[:])
            nc.sync.dma_start(out_d[128 * sblk:128 * (sblk + 1), :], osb[:])

        # ---- emission order (== program order for tile deps): vaug[s]
        # must be written before the attention ib that reads it; attnT
        # before the out-proj s-blocks that read it. exp work starts as
        # early as possible; out-proj interleaves with the last pair. -----
        emit_qk(0)
        for ib in range(NIB):
            for sblk in range(4 * ib, 4 * ib + 4):
                emit_v(sblk)
            emit_attn(0, only_ib=ib)
        emit_qk(1)
        emit_attn(1)
        emit_qk(2)
        emit_attn(2)
        emit_qk(3)
        for ib in range(NIB):
            emit_attn(3, only_ib=ib)
            for sblk in range(4 * ib, 4 * ib + 4):
                emit_outproj(sblk)

    nc.compile()
    return nc


def _get_nc():
    if "nc" not in _CACHE:
        _CACHE["nc"] = _build()
    return _CACHE["nc"]


def _shard_inputs(x, w_qkv, w_out):
    bf = ml_dtypes.bfloat16
    in_maps = []
    for c in range(N_CORES):
        b, g = divmod(c, 2)
        xT = np.ascontiguousarray(x[b].T).astype(bf)
        wq = w_qkv[512 * g:512 * (g + 1)]
        wk = w_qkv[1024 + 512 * g:1024 + 512 * (g + 1)]
        wv = w_qkv[2048 + 512 * g:2048 + 512 * (g + 1)]
        wqkvT = np.ascontiguousarray(
            np.concatenate([wq, wk, wv], axis=0).T).astype(bf)
        woutT = np.ascontiguousarray(w_out[:, 512 * g:512 * (g + 1)].T
                                     ).astype(bf)
        in_maps.append({"xT": xT, "wqkvT": wqkvT, "woutT": woutT})
    return in_maps


def kernel(x, w_qkv, w_out):
    global LAST_EXEC_TIME_NS
    from concourse.bass_utils import run_bass_kernel_spmd

    nc = _get_nc()
    in_maps = _shard_inputs(np.asarray(x, dtype=np.float32),
                            np.asarray(w_qkv, dtype=np.float32),
                            np.asarray(w_out, dtype=np.float32))
    trace = bool(int(os.environ.get("KBENCH_TRACE", "0")))
    res = run_bass_kernel_spmd(nc, in_maps, list(range(N_CORES)), trace=trace)
    LAST_EXEC_TIME_NS = res.exec_time_ns
    out = np.empty((4, S, D), dtype=np.float32)
    for b in range(4):
        out[b] = res.results[2 * b]["out"] + res.results[2 * b + 1]["out"]
    return out


# revision 21
# speedup vs baseline: 1.9745x; 1.0103x over previous
"""Causal self-attention (B=4, S=2048, D=1024, H=16, hd=64) on 8 TRN2 cores.

Sharding: core c = (batch b = c//2, head-group g = c%2); each core computes
8 heads for one batch. Out-projection partials are summed on host (the only
cross-shard reduction).

Device kernel layout (all matmul contractions have the contracted dim on
SBUF partitions; everything stays transposed so no on-device transposes):
  qT,kT  [64*2heads, S]  = wqkvT-chunk.T @ xT          (stationary weights)
  v_aug  [S-block, 8*65] = xT-chunk.T @ wvT (+ ones col per head for sums)
  sT     [j 128, i 512]  = kT-slice.T @ qT-slice        (2 heads row-packed)
  pT     = exp(sT/8)  bf16 via ACT; diagonal blocks masked by affine_select
  outT   [65, i]        += v_aug.T @ pT   (row 64 accumulates softmax sums)
  attnT  = outT * bcast(1/sums)           (gpsimd partition_broadcast)
  out    [s 128, e]      = attnT-chunk.T @ woutT-chunk  (accum over c-chunks)

QKV work for pair p+1 is emitted between attention pairs so the PE always
has independent matmuls while ACT runs exp (keeps HAM at full clock).
"""
import sys
import os

sys.path.insert(0, "/opt/trn_rl_repo")

import numpy as np
import ml_dtypes
from contextlib import ExitStack

S = 2048
D = 1024
HL = 8          # heads per core
HD = 64
PAIRS = 4       # head pairs per core
NIB = 4         # i-blocks of 512
N_CORES = 8

_CACHE = {}
LAST_EXEC_TIME_NS = None


def _build():
    import concourse.tile as tile
    import concourse.mybir as mybir
    from concourse import bacc

    bf = mybir.dt.bfloat16
    f32 = mybir.dt.float32
    EXP = mybir.ActivationFunctionType.Exp
    GE = mybir.AluOpType.is_ge

    nc = bacc.Bacc("TRN2", target_bir_lowering=False, debug=False,
                   num_devices=N_CORES)
    xT_d = nc.dram_tensor("xT", [D, S], bf, kind="ExternalInput").ap()
    wqkvT_d = nc.dram_tensor("wqkvT", [D, 3 * 512], bf,
                             kind="ExternalInput").ap()
    woutT_d = nc.dram_tensor("woutT", [512, D], bf, kind="ExternalInput").ap()
    out_d = nc.dram_tensor("out", [S, D], f32, kind="ExternalOutput").ap()

    with tile.TileContext(nc) as tc, ExitStack() as ctx:
        sb = ctx.enter_context(tc.tile_pool(name="sb", bufs=1))
        # PSUM: "mm" = 2x [128,1024] (score batches), "ps5" = 4x [128,512]
        # (qkv accum, AV accum, out-proj accum) -> 8 banks total.
        mm = ctx.enter_context(tc.tile_pool(name="mm", bufs=2, space="PSUM"))
        ps5 = ctx.enter_context(tc.tile_pool(name="ps5", bufs=4,
                                             space="PSUM"))
        pp = ctx.enter_context(tc.tile_pool(name="pp", bufs=6))
        rsp = ctx.enter_context(tc.tile_pool(name="rsp", bufs=4))
        bcsp = ctx.enter_context(tc.tile_pool(name="bcsp", bufs=4))
        osbp = ctx.enter_context(tc.tile_pool(name="osbp", bufs=2))

        # ---- persistent SBUF tiles -------------------------------------
        xt = [sb.tile([128, S], bf, tag=f"xt{d}", name=f"xt{d}")
              for d in range(8)]
        wqkv = [sb.tile([128, 1536], bf, tag=f"wqkv{d}", name=f"wqkv{d}")
                for d in range(8)]
        wout = [sb.tile([128, D], bf, tag=f"wout{c}", name=f"wout{c}")
                for c in range(4)]
        qT = [sb.tile([128, S], bf, tag=f"qT{p}", name=f"qT{p}")
              for p in range(PAIRS)]
        kT = [sb.tile([128, S], bf, tag=f"kT{p}", name=f"kT{p}")
              for p in range(PAIRS)]
        vaug = [sb.tile([128, HL, HD + 1], bf, tag=f"vaug{s}",
                        name=f"vaug{s}") for s in range(16)]
        attnT = [sb.tile([128, S], bf, tag=f"attnT{p}", name=f"attnT{p}")
                 for p in range(PAIRS)]

        for d in range(8):
            nc.sync.dma_start(xt[d][:], xT_d[128 * d:128 * (d + 1), :])
            nc.sync.dma_start(wqkv[d][:], wqkvT_d[128 * d:128 * (d + 1), :])
        for c in range(4):
            nc.sync.dma_start(wout[c][:], woutT_d[128 * c:128 * (c + 1), :])
        for s in range(16):
            nc.gpsimd.memset(vaug[s][:], 1.0)
        # causal masks for the 4 diagonal offsets: keep where i >= 128*m + j
        masks = [sb.tile([128, 512], bf, tag=f"mask{m}", name=f"mask{m}")
                 for m in range(4)]
        for m in range(4):
            nc.gpsimd.memset(masks[m][:], 1.0)
            nc.gpsimd.affine_select(
                out=masks[m][:], in_=masks[m][:], compare_op=GE, fill=0.0,
                base=-128 * m, channel_multiplier=-1, pattern=[[1, 512]])

        # ---- emission helpers ------------------------------------------
        def emit_v(sblk):
            ps = ps5.tile([128, 512], f32, tag="ps5", name=f"vps{sblk}")
            for dc in range(8):
                nc.tensor.matmul(ps[:],
                                 lhsT=xt[dc][:, 128 * sblk:128 * (sblk + 1)],
                                 rhs=wqkv[dc][:, 1024:1536],
                                 start=(dc == 0), stop=(dc == 7))
            nc.vector.tensor_copy(
                vaug[sblk][:, :, 0:64],
                ps[:].rearrange("p (h d) -> p h d", h=HL))

        def emit_qk(pair):
            # nb = pair -> q columns, nb = pair + 4 -> k columns
            for nb in (pair, pair + 4):
                dest = qT[pair] if nb < 4 else kT[pair]
                for sc in range(4):
                    ps = ps5.tile([128, 512], f32, tag="ps5",
                                  name=f"qkps{nb}_{sc}")
                    for dc in range(8):
                        nc.tensor.matmul(
                            ps[:],
                            lhsT=wqkv[dc][:, 128 * nb:128 * (nb + 1)],
                            rhs=xt[dc][:, 512 * sc:512 * (sc + 1)],
                            start=(dc == 0), stop=(dc == 7))
                    nc.vector.tensor_copy(dest[:, 512 * sc:512 * (sc + 1)],
                                          ps[:])

        def emit_qkexp(pair, ib, jb):
            off = max(0, 128 * (jb - 4 * ib))
            s2 = mm.tile([128, 1024], f32, tag="mm",
                         name=f"s2_{pair}{ib}{jb}")
            for h01 in range(2):
                r0, r1 = 64 * h01, 64 * (h01 + 1)
                nc.tensor.matmul(
                    s2[:, 512 * h01 + off:512 * (h01 + 1)],
                    lhsT=kT[pair][r0:r1, 128 * jb:128 * (jb + 1)],
                    rhs=qT[pair][r0:r1, 512 * ib + off:512 * (ib + 1)],
                    start=True, stop=True)
            pX = pp.tile([128, 1024], bf, tag="pp", name=f"pX{pair}{ib}{jb}")
            s3 = s2[:].rearrange("p (h i) -> p h i", h=2)
            p3 = pX[:].rearrange("p (h i) -> p h i", h=2)
            nc.scalar.activation(p3[:, :, off:512], s3[:, :, off:512],
                                 EXP, scale=0.125)
            if jb >= 4 * ib:
                m = jb - 4 * ib
                nc.vector.tensor_mul(
                    p3[:, :, off:512], p3[:, :, off:512],
                    masks[m][:, off:512].unsqueeze(1).broadcast_to(
                        [128, 2, 512 - off]))
            return pX

        def emit_attn(pair, only_ib=None, pre_px=None):
            for ib in range(NIB) if only_ib is None else [only_ib]:
                n_jb = 4 * (ib + 1)
                oA = ps5.tile([65, 512], f32, tag="ps5", name=f"oA{pair}{ib}")
                oB = ps5.tile([65, 512], f32, tag="ps5", name=f"oB{pair}{ib}")
                for jb in range(n_jb):
                    off = max(0, 128 * (jb - 4 * ib))
                    if pre_px is not None and jb in pre_px:
                        pX = pre_px[jb]
                    else:
                        pX = emit_qkexp(pair, ib, jb)
                    for h01, oX in ((0, oA), (1, oB)):
                        nc.tensor.matmul(
                            oX[:, off:512],
                            lhsT=vaug[jb][:, 2 * pair + h01, :],
                            rhs=pX[:, 512 * h01 + off:512 * (h01 + 1)],
                            start=(jb == 0), stop=(jb == n_jb - 1))
                for h01, oX in ((0, oA), (1, oB)):
                    tmp = rsp.tile([1, 512], f32, tag="rtmp",
                                   name=f"rt{pair}{ib}{h01}")
                    nc.vector.tensor_copy(tmp[:], oX[64:65, :])
                    rs = rsp.tile([1, 512], f32, tag="rsp",
                                  name=f"rs{pair}{ib}{h01}")
                    nc.vector.reciprocal_approx_fast(rs[:], tmp[:])
                    bcs = bcsp.tile([64, 512], f32, tag="bcsp",
                                    name=f"bcs{pair}{ib}{h01}")
                    nc.gpsimd.partition_broadcast(bcs[:], rs[:])
                    nc.vector.tensor_mul(
                        attnT[pair][64 * h01:64 * (h01 + 1),
                                    512 * ib:512 * (ib + 1)],
                        oX[0:64, :], bcs[:])

        def emit_outproj(sblk):
            osb = osbp.tile([128, D], f32, tag="osbp", name=f"osb{sblk}")
            for eh in range(2):
                ps = ps5.tile([128, 512], f32, tag="ps5",
                              name=f"ops{sblk}{eh}")
                for cc in range(4):
                    nc.tensor.matmul(
                        ps[:],
                        lhsT=attnT[cc][:, 128 * sblk:128 * (sblk + 1)],
                        rhs=wout[cc][:, 512 * eh:512 * (eh + 1)],
                        start=(cc == 0), stop=(cc == 3))
                nc.scalar.copy(osb[:, 512 * eh:512 * (eh + 1)], ps[:])
            nc.sync.dma_start(out_d[128 * sblk:128 * (sblk + 1), :], osb[:])

        # ---- emission order (== program order for tile deps): vaug[s]
        # must be written before the attention ib that reads it; attnT
        # before the out-proj s-blocks that read it. exp work starts as
        # early as possible; out-proj interleaves with the last pair. -----
        emit_qk(0)
        # ib0 of pair 0: QK+exp emitted before the v-phase so ACT starts
        # as early as possible (AV waits for vaug, exp does not)
        pre = {jb: emit_qkexp(0, 0, jb) for jb in range(4)}
        for sblk in range(4):
            emit_v(sblk)
        emit_attn(0, only_ib=0, pre_px=pre)
        for ib in range(1, NIB):
            for sblk in range(4 * ib, 4 * ib + 4):
                emit_v(sblk)
            emit_attn(0, only_ib=ib)
        emit_qk(1)
        emit_attn(1)
        emit_qk(2)
        emit_attn(2)
        emit_qk(3)
        for ib in range(NIB):
            emit_attn(3, only_ib=ib)
            for sblk in range(4 * ib, 4 * ib + 4):
                emit_outproj(sblk)

    nc.compile()
    return nc


def _get_nc():
    if "nc" not in _CACHE:
        _CACHE["nc"] = _build()
    return _CACHE["nc"]


def _shard_inputs(x, w_qkv, w_out):
    bf = ml_dtypes.bfloat16
    in_maps = []
    for c in range(N_CORES):
        b, g = divmod(c, 2)
        xT = np.ascontiguousarray(x[b].T).astype(bf)
        wq = w_qkv[512 * g:512 * (g + 1)]
        wk = w_qkv[1024 + 512 * g:1024 + 512 * (g + 1)]
        wv = w_qkv[2048 + 512 * g:2048 + 512 * (g + 1)]
        wqkvT = np.ascontiguousarray(
            np.concatenate([wq, wk, wv], axis=0).T).astype(bf)
        woutT = np.ascontiguousarray(w_out[:, 512 * g:512 * (g + 1)].T
                                     ).astype(bf)
        in_maps.append({"xT": xT, "wqkvT": wqkvT, "woutT": woutT})
    return in_maps


def kernel(x, w_qkv, w_out):
    global LAST_EXEC_TIME_NS
    from concourse.bass_utils import run_bass_kernel_spmd

    nc = _get_nc()
    in_maps = _shard_inputs(np.asarray(x, dtype=np.float32),
                            np.asarray(w_qkv, dtype=np.float32),
                            np.asarray(w_out, dtype=np.float32))
    trace = bool(int(os.environ.get("KBENCH_TRACE", "0")))
    res = run_bass_kernel_spmd(nc, in_maps, list(range(N_CORES)), trace=trace)
    LAST_EXEC_TIME_NS = res.exec_time_ns
    out = np.empty((4, S, D), dtype=np.float32)
    for b in range(4):
        out[b] = res.results[2 * b]["out"] + res.results[2 * b + 1]["out"]
    return out


# revision 22
# speedup vs baseline: 1.9792x; 1.0024x over previous
"""Causal self-attention (B=4, S=2048, D=1024, H=16, hd=64) on 8 TRN2 cores.

Sharding: core c = (batch b = c//2, head-group g = c%2); each core computes
8 heads for one batch. Out-projection partials are summed on host (the only
cross-shard reduction).

Device kernel layout (all matmul contractions have the contracted dim on
SBUF partitions; everything stays transposed so no on-device transposes):
  qT,kT  [64*2heads, S]  = wqkvT-chunk.T @ xT          (stationary weights)
  v_aug  [S-block, 8*65] = xT-chunk.T @ wvT (+ ones col per head for sums)
  sT     [j 128, i 512]  = kT-slice.T @ qT-slice        (2 heads row-packed)
  pT     = exp(sT/8)  bf16 via ACT; diagonal blocks masked by affine_select
  outT   [65, i]        += v_aug.T @ pT   (row 64 accumulates softmax sums)
  attnT  = outT * bcast(1/sums)           (gpsimd partition_broadcast)
  out    [s 128, e]      = attnT-chunk.T @ woutT-chunk  (accum over c-chunks)

QKV work for pair p+1 is emitted between attention pairs so the PE always
has independent matmuls while ACT runs exp (keeps HAM at full clock).
"""
import sys
import os

sys.path.insert(0, "/opt/trn_rl_repo")

import numpy as np
import ml_dtypes
from contextlib import ExitStack

S = 2048
D = 1024
HL = 8          # heads per core
HD = 64
PAIRS = 4       # head pairs per core
NIB = 4         # i-blocks of 512
N_CORES = 8

_CACHE = {}
LAST_EXEC_TIME_NS = None


def _build():
    import concourse.tile as tile
    import concourse.mybir as mybir
    from concourse import bacc

    bf = mybir.dt.bfloat16
    f32 = mybir.dt.float32
    EXP = mybir.ActivationFunctionType.Exp
    GE = mybir.AluOpType.is_ge

    nc = bacc.Bacc("TRN2", target_bir_lowering=False, debug=False,
                   num_devices=N_CORES)
    xT_d = nc.dram_tensor("xT", [D, S], bf, kind="ExternalInput").ap()
    wqkvT_d = nc.dram_tensor("wqkvT", [D, 3 * 512], bf,
                             kind="ExternalInput").ap()
    woutT_d = nc.dram_tensor("woutT", [512, D], bf, kind="ExternalInput").ap()
    out_d = nc.dram_tensor("out", [S, D], f32, kind="ExternalOutput").ap()

    with tile.TileContext(nc) as tc, ExitStack() as ctx:
        sb = ctx.enter_context(tc.tile_pool(name="sb", bufs=1))
        # PSUM: "mm" = 2x [128,1024] (score batches), "ps5" = 4x [128,512]
        # (qkv accum, AV accum, out-proj accum) -> 8 banks total.
        mm = ctx.enter_context(tc.tile_pool(name="mm", bufs=2, space="PSUM"))
        ps5 = ctx.enter_context(tc.tile_pool(name="ps5", bufs=4,
                                             space="PSUM"))
        pp = ctx.enter_context(tc.tile_pool(name="pp", bufs=8))
        rsp = ctx.enter_context(tc.tile_pool(name="rsp", bufs=4))
        bcsp = ctx.enter_context(tc.tile_pool(name="bcsp", bufs=4))
        osbp = ctx.enter_context(tc.tile_pool(name="osbp", bufs=3))

        # ---- persistent SBUF tiles -------------------------------------
        xt = [sb.tile([128, S], bf, tag=f"xt{d}", name=f"xt{d}")
              for d in range(8)]
        wqkv = [sb.tile([128, 1536], bf, tag=f"wqkv{d}", name=f"wqkv{d}")
                for d in range(8)]
        wout = [sb.tile([128, D], bf, tag=f"wout{c}", name=f"wout{c}")
                for c in range(4)]
        qT = [sb.tile([128, S], bf, tag=f"qT{p}", name=f"qT{p}")
              for p in range(PAIRS)]
        kT = [sb.tile([128, S], bf, tag=f"kT{p}", name=f"kT{p}")
              for p in range(PAIRS)]
        vaug = [sb.tile([128, HL, HD + 1], bf, tag=f"vaug{s}",
                        name=f"vaug{s}") for s in range(16)]
        attnT = [sb.tile([128, S], bf, tag=f"attnT{p}", name=f"attnT{p}")
                 for p in range(PAIRS)]

        for d in range(8):
            nc.sync.dma_start(xt[d][:], xT_d[128 * d:128 * (d + 1), :])
            nc.sync.dma_start(wqkv[d][:], wqkvT_d[128 * d:128 * (d + 1), :])
        for c in range(4):
            nc.sync.dma_start(wout[c][:], woutT_d[128 * c:128 * (c + 1), :])
        for s in range(16):
            nc.gpsimd.memset(vaug[s][:], 1.0)
        # causal masks for the 4 diagonal offsets: keep where i >= 128*m + j
        masks = [sb.tile([128, 512], bf, tag=f"mask{m}", name=f"mask{m}")
                 for m in range(4)]
        for m in range(4):
            nc.gpsimd.memset(masks[m][:], 1.0)
            nc.gpsimd.affine_select(
                out=masks[m][:], in_=masks[m][:], compare_op=GE, fill=0.0,
                base=-128 * m, channel_multiplier=-1, pattern=[[1, 512]])

        # ---- emission helpers ------------------------------------------
        def emit_v(sblk):
            ps = ps5.tile([128, 512], f32, tag="ps5", name=f"vps{sblk}")
            for dc in range(8):
                nc.tensor.matmul(ps[:],
                                 lhsT=xt[dc][:, 128 * sblk:128 * (sblk + 1)],
                                 rhs=wqkv[dc][:, 1024:1536],
                                 start=(dc == 0), stop=(dc == 7))
            nc.scalar.copy(
                vaug[sblk][:, :, 0:64],
                ps[:].rearrange("p (h d) -> p h d", h=HL))

        def emit_qk(pair):
            # nb = pair -> q columns, nb = pair + 4 -> k columns
            for nb in (pair, pair + 4):
                dest = qT[pair] if nb < 4 else kT[pair]
                for sc in range(4):
                    ps = ps5.tile([128, 512], f32, tag="ps5",
                                  name=f"qkps{nb}_{sc}")
                    for dc in range(8):
                        nc.tensor.matmul(
                            ps[:],
                            lhsT=wqkv[dc][:, 128 * nb:128 * (nb + 1)],
                            rhs=xt[dc][:, 512 * sc:512 * (sc + 1)],
                            start=(dc == 0), stop=(dc == 7))
                    nc.vector.tensor_copy(dest[:, 512 * sc:512 * (sc + 1)],
                                          ps[:])

        def emit_qkexp(pair, ib, jb):
            off = max(0, 128 * (jb - 4 * ib))
            s2 = mm.tile([128, 1024], f32, tag="mm",
                         name=f"s2_{pair}{ib}{jb}")
            for h01 in range(2):
                r0, r1 = 64 * h01, 64 * (h01 + 1)
                nc.tensor.matmul(
                    s2[:, 512 * h01 + off:512 * (h01 + 1)],
                    lhsT=kT[pair][r0:r1, 128 * jb:128 * (jb + 1)],
                    rhs=qT[pair][r0:r1, 512 * ib + off:512 * (ib + 1)],
                    start=True, stop=True)
            pX = pp.tile([128, 1024], bf, tag="pp", name=f"pX{pair}{ib}{jb}")
            s3 = s2[:].rearrange("p (h i) -> p h i", h=2)
            p3 = pX[:].rearrange("p (h i) -> p h i", h=2)
            nc.scalar.activation(p3[:, :, off:512], s3[:, :, off:512],
                                 EXP, scale=0.125)
            if jb >= 4 * ib:
                m = jb - 4 * ib
                nc.vector.tensor_mul(
                    p3[:, :, off:512], p3[:, :, off:512],
                    masks[m][:, off:512].unsqueeze(1).broadcast_to(
                        [128, 2, 512 - off]))
            return pX

        def emit_attn(pair, only_ib=None, pre_px=None):
            for ib in range(NIB) if only_ib is None else [only_ib]:
                n_jb = 4 * (ib + 1)
                oA = ps5.tile([65, 512], f32, tag="ps5", name=f"oA{pair}{ib}")
                oB = ps5.tile([65, 512], f32, tag="ps5", name=f"oB{pair}{ib}")
                for jb in range(n_jb):
                    off = max(0, 128 * (jb - 4 * ib))
                    if pre_px is not None and jb in pre_px:
                        pX = pre_px[jb]
                    else:
                        pX = emit_qkexp(pair, ib, jb)
                    for h01, oX in ((0, oA), (1, oB)):
                        nc.tensor.matmul(
                            oX[:, off:512],
                            lhsT=vaug[jb][:, 2 * pair + h01, :],
                            rhs=pX[:, 512 * h01 + off:512 * (h01 + 1)],
                            start=(jb == 0), stop=(jb == n_jb - 1))
                for h01, oX in ((0, oA), (1, oB)):
                    tmp = rsp.tile([1, 512], f32, tag="rtmp",
                                   name=f"rt{pair}{ib}{h01}")
                    nc.vector.tensor_copy(tmp[:], oX[64:65, :])
                    rs = rsp.tile([1, 512], f32, tag="rsp",
                                  name=f"rs{pair}{ib}{h01}")
                    nc.vector.reciprocal_approx_fast(rs[:], tmp[:])
                    bcs = bcsp.tile([64, 512], f32, tag="bcsp",
                                    name=f"bcs{pair}{ib}{h01}")
                    nc.gpsimd.partition_broadcast(bcs[:], rs[:])
                    nc.vector.tensor_mul(
                        attnT[pair][64 * h01:64 * (h01 + 1),
                                    512 * ib:512 * (ib + 1)],
                        oX[0:64, :], bcs[:])

        def emit_outproj(sblk):
            osb = osbp.tile([128, D], f32, tag="osbp", name=f"osb{sblk}")
            for eh in range(2):
                ps = ps5.tile([128, 512], f32, tag="ps5",
                              name=f"ops{sblk}{eh}")
                for cc in range(4):
                    nc.tensor.matmul(
                        ps[:],
                        lhsT=attnT[cc][:, 128 * sblk:128 * (sblk + 1)],
                        rhs=wout[cc][:, 512 * eh:512 * (eh + 1)],
                        start=(cc == 0), stop=(cc == 3))
                nc.scalar.copy(osb[:, 512 * eh:512 * (eh + 1)], ps[:])
            nc.sync.dma_start(out_d[128 * sblk:128 * (sblk + 1), :], osb[:])

        # ---- emission order (== program order for tile deps): vaug[s]
        # must be written before the attention ib that reads it; attnT
        # before the out-proj s-blocks that read it. exp work starts as
        # early as possible; out-proj interleaves with the last pair. -----
        emit_qk(0)
        # ib0 of pair 0: QK+exp emitted before the v-phase so ACT starts
        # as early as possible (AV waits for vaug, exp does not)
        pre = {jb: emit_qkexp(0, 0, jb) for jb in range(4)}
        for sblk in range(4):
            emit_v(sblk)
        emit_attn(0, only_ib=0, pre_px=pre)
        for ib in range(1, NIB):
            for sblk in range(4 * ib, 4 * ib + 4):
                emit_v(sblk)
            emit_attn(0, only_ib=ib)
        emit_qk(1)
        emit_attn(1)
        emit_qk(2)
        emit_attn(2)
        emit_qk(3)
        for ib in range(NIB):
            emit_attn(3, only_ib=ib)
            for sblk in range(4 * ib, 4 * ib + 4):
                emit_outproj(sblk)

    nc.compile()
    return nc


def _get_nc():
    if "nc" not in _CACHE:
        _CACHE["nc"] = _build()
    return _CACHE["nc"]


def _shard_inputs(x, w_qkv, w_out):
    bf = ml_dtypes.bfloat16
    in_maps = []
    for c in range(N_CORES):
        b, g = divmod(c, 2)
        xT = np.ascontiguousarray(x[b].T).astype(bf)
        wq = w_qkv[512 * g:512 * (g + 1)]
        wk = w_qkv[1024 + 512 * g:1024 + 512 * (g + 1)]
        wv = w_qkv[2048 + 512 * g:2048 + 512 * (g + 1)]
        wqkvT = np.ascontiguousarray(
            np.concatenate([wq, wk, wv], axis=0).T).astype(bf)
        woutT = np.ascontiguousarray(w_out[:, 512 * g:512 * (g + 1)].T
                                     ).astype(bf)
        in_maps.append({"xT": xT, "wqkvT": wqkvT, "woutT": woutT})
    return in_maps


def kernel(x, w_qkv, w_out):
    global LAST_EXEC_TIME_NS
    from concourse.bass_utils import run_bass_kernel_spmd

    nc = _get_nc()
    in_maps = _shard_inputs(np.asarray(x, dtype=np.float32),
                            np.asarray(w_qkv, dtype=np.float32),
                            np.asarray(w_out, dtype=np.float32))
    trace = bool(int(os.environ.get("KBENCH_TRACE", "0")))
    res = run_bass_kernel_spmd(nc, in_maps, list(range(N_CORES)), trace=trace)
    LAST_EXEC_TIME_NS = res.exec_time_ns
    out = np.empty((4, S, D), dtype=np.float32)
    for b in range(4):
        out[b] = res.results[2 * b]["out"] + res.results[2 * b + 1]["out"]
    return out
